# revision 13
# baseline (speedup 1.0000x reference)
import sys

sys.path.insert(0, "/opt/trn_rl_repo")
import numpy as np

N1, N2, D = 8192, 8192, 256
NCORES = 8
QPC = N1 // NCORES  # queries per core (1024)
RT = QPC // 128  # row tiles per core (8)
GW = 2048  # colgroup width (4 psum banks)
NCG = 8  # colgroups (4 per side)

# side-1 (nu) colgroups are staged: ACT copies psum->fp16, norms added by
# GpSimd TT-add (cg 4,5,6) or PE rank-2 matmuls (cg 7), maxes accumulated
# on DVE via fp16 tensor_tensor (2x mode), one stock reduce per query tile.
STAGED = (4, 5, 6, 7)
GP_NORM = (4, 5, 6)  # staged cgs whose norm-add runs on GpSimd
FIRST_STAGED = 4

_OP_NAME = "TT_ADD_MAX_DIAG_ANT"


def _register_custom_op():
    """Fused DVE op: body = (idx==c0 ? -FLT_MAX : in0 + in1), accum = max."""
    import concourse.dve_ops as dve_ops

    for op in dve_ops.OPS:
        if op.name == _OP_NAME:
            return op
    from concourse.dve_spec import (
        C0,
        Idx,
        MaxNeg,
        Spec,
        Src0,
        Src1,
        eq,
        lower,
        maxx,
        select,
        _has_src1,
    )
    from concourse.dve_uop import DveOpSpec

    def _ref(in0, in1, c0, c1, c2):
        P = in0.shape[0]
        x = in0.astype(np.float32).reshape(P, -1)
        y = np.broadcast_to(np.asarray(in1, np.float32).reshape(P, -1), x.shape)
        n = x.shape[1]
        idx = np.broadcast_to(np.arange(n, dtype=np.float32), (P, n))
        c0b = np.broadcast_to(np.asarray(c0, np.float32).reshape(-1, 1), (P, 1))
        fmin = np.float32(np.finfo(np.float32).min)
        body = np.where(idx == c0b, fmin, x + y).astype(np.float32)
        acc = np.maximum(np.max(body, axis=-1, keepdims=True), fmin)
        return body, acc

    spec = Spec(
        body=select(eq(Idx, C0), MaxNeg, Src0 + Src1),
        accum=maxx,
        reference=_ref,
    )
    row = dve_ops._CUSTOM_DVE_ROW_BASE + len(dve_ops.OPS)
    shas = {}
    for ver in ("v3", "v4"):
        try:
            u = lower(spec, ver=ver)
            shas[ver] = DveOpSpec(
                name=_OP_NAME, opcode=row, uops=u, rd1_en=_has_src1(spec)
            ).sha(ver)
        except ValueError:
            pass
    op = dve_ops.DveOp(_OP_NAME, spec, subdim=False, uops_sha=shas)
    dve_ops.OPS.append(op)
    dve_ops._SUB_OPCODE_FOR_NAME[_OP_NAME] = row
    dve_ops.CUSTOM_DVE_SPECS[_OP_NAME] = spec
    return op


def _build_nc():
    import concourse.bass as bass
    import concourse.tile as tile
    from concourse import mybir

    ttop = _register_custom_op()

    f32 = mybir.dt.float32
    bf16 = mybir.dt.bfloat16
    fp16 = mybir.dt.float16
    max_op = mybir.AluOpType.max
    add_op = mybir.AluOpType.add

    nc = bass.Bass()
    # cg0 split by contraction half so the two DMA queues race it in first
    db0kd = [
        nc.dram_tensor(f"db0k{k}", [128, GW], bf16, kind="ExternalInput")
        for k in (0, 1)
    ]
    dbd = [None] + [
        nc.dram_tensor(f"db{c}", [128, 2, GW], bf16, kind="ExternalInput")
        for c in range(1, NCG)
    ]
    nbcd = [
        nc.dram_tensor(f"nbc{c}", [128, GW], fp16, kind="ExternalInput")
        for c in range(NCG)
    ]
    # bf16 hi/lo rows of side-1 -0.5||y||^2 plus trailing ones block
    nrmhd = nc.dram_tensor("nrmh", [2, N1 + 128], bf16, kind="ExternalInput")
    diagwd = nc.dram_tensor("diagw", [128, RT], f32, kind="ExternalInput")
    o = nc.dram_tensor("o", [128, RT, 5], f32, kind="ExternalOutput")

    with tile.TileContext(nc) as tc:
        with (
            tc.tile_pool(name="sb", bufs=1) as sb,
            tc.tile_pool(name="pp", bufs=1) as pp,
            tc.tile_pool(name="st", bufs=3) as st,
            tc.tile_pool(name="ps", bufs=2, space="PSUM") as ps,
        ):
            # DMA plan: small/critical first; db/nbc chunks land in
            # consumption order, spread over the three DGE queues.
            tdw = sb.tile([128, RT], f32, tag="dw")
            nc.gpsimd.dma_start(out=tdw, in_=diagwd[:])
            tnbc = [
                sb.tile([128, GW], fp16, name=f"nbc{c}", tag=f"nbc{c}")
                for c in range(NCG)
            ]
            nc.gpsimd.dma_start(out=tnbc[0], in_=nbcd[0][:])
            tnh = sb.tile([2, N1 + 128], bf16, tag="nh")
            nc.gpsimd.dma_start(out=tnh, in_=nrmhd[:])
            qs = [nc.sync, nc.scalar]
            tdb0k = [
                sb.tile([128, GW], bf16, name=f"db0k{k}", tag=f"db0k{k}")
                for k in (0, 1)
            ]
            nc.sync.dma_start(out=tdb0k[0], in_=db0kd[0][:])
            nc.scalar.dma_start(out=tdb0k[1], in_=db0kd[1][:])
            tdb = [None] + [
                sb.tile([128, 2, GW], bf16, name=f"db{c}", tag=f"db{c}")
                for c in range(1, NCG)
            ]
            for c in range(1, NCG):
                qs[c % 2].dma_start(out=tdb[c], in_=dbd[c][:])
            for c in range(1, NCG):
                nc.gpsimd.dma_start(out=tnbc[c], in_=nbcd[c][:])

            # wait absorbers: DVE observes the startup-critical DMAs once
            dum = sb.tile([128, 2], f32, tag="dum")
            nc.vector.tensor_copy(out=dum[:, 0:1], in_=tdw[:, 0:1])
            nc.vector.tensor_copy(out=dum[:, 1:2], in_=tnbc[0][:, 0:1])
            dumo = sb.tile([128, 1], f32, tag="dumo")

            ones2 = tnh[0:2, N1 : N1 + 128]
            parts = [
                pp.tile([128, 5], f32, name=f"part{m}", tag=f"part{m}")
                for m in range(RT)
            ]
            accs = [
                pp.tile([128, GW], fp16, name=f"acc{m}", tag=f"acc{m}")
                for m in range(RT)
            ]
            for cg in range(NCG):
                for m in range(RT):
                    lhs = [tdb0k[k][:, m * 128 : (m + 1) * 128] for k in (0, 1)]
                    pst = ps.tile([128, GW], f32, tag="pst")
                    pe_norm = cg in STAGED and cg not in GP_NORM
                    for k in (0, 1):
                        for i in (0, 1, 2, 3):
                            rhs = (
                                tdb0k[k][:, i * 512 : (i + 1) * 512]
                                if cg == 0
                                else tdb[cg][:, k, i * 512 : (i + 1) * 512]
                            )
                            nc.tensor.matmul(
                                out=pst[:, i * 512 : (i + 1) * 512],
                                lhsT=lhs[k],
                                rhs=rhs,
                                start=(k == 0),
                                stop=(k == 1 and not pe_norm),
                            )
                    if pe_norm:
                        nb = (cg - 4) * GW
                        for i in (0, 1, 2, 3):
                            nc.tensor.matmul(
                                out=pst[:, i * 512 : (i + 1) * 512],
                                lhsT=ones2,
                                rhs=tnh[0:2, nb + i * 512 : nb + (i + 1) * 512],
                                start=False,
                                stop=True,
                            )
                    if cg not in STAGED:
                        nc.vector._custom_dve(
                            ttop,
                            out=dumo.broadcast_to(pst.shape),
                            in0=pst,
                            in1=tnbc[cg][:, :],
                            s0=tdw[:, m : m + 1] if cg == 0 else -1.0,
                            s1=0.0,
                            imm2=0.0,
                            accum_out=parts[m][:, cg : cg + 1],
                        )
                        continue
                    if cg == FIRST_STAGED:
                        tgt = accs[m]
                    else:
                        tgt = st.tile([128, GW], fp16, tag="stg")
                    nc.scalar.copy(out=tgt, in_=pst)
                    if cg in GP_NORM:
                        nc.gpsimd.tensor_tensor(
                            out=tgt, in0=tgt, in1=tnbc[cg][:, :], op=add_op
                        )
                    if cg != FIRST_STAGED:
                        nc.vector.tensor_tensor(
                            out=accs[m], in0=accs[m], in1=tgt, op=max_op
                        )
                    if cg == NCG - 1:
                        nc.vector.tensor_reduce(
                            out=parts[m][:, 4:5],
                            in_=accs[m],
                            axis=mybir.AxisListType.X,
                            op=max_op,
                        )
            for m in range(RT):
                nc.sync.dma_start(out=o[:, m, :], in_=parts[m])

    from concourse.bass import _bass_rust
    from concourse.library_overlay import lower_extended_insts

    lower_extended_insts(nc)  # populate .instr for InstISA subclasses
    _bass_rust.move_matmul_waits_to_ldweights(nc.m)
    _bass_rust.generate_event_semaphores(nc)
    return nc


def _bf16_hilo(x64):
    import ml_dtypes

    hi = x64.astype(np.float32).astype(ml_dtypes.bfloat16)
    lo = (x64 - hi.astype(np.float64)).astype(np.float32).astype(ml_dtypes.bfloat16)
    return hi, lo


def _prep_core(s1, s2T_bf, c2_half, c):
    import ml_dtypes

    bf = ml_dtypes.bfloat16
    s1p = np.roll(s1, -c * QPC, axis=0)
    s1pT_bf = np.ascontiguousarray(s1p.T).astype(bf)
    dbx = np.empty((128, 2, 2 * N1), dtype=bf)
    for k in (0, 1):
        dbx[:, k, 0:N1] = s1pT_bf[k * 128 : (k + 1) * 128]
        dbx[:, k, N1 : 2 * N1] = s2T_bf[k * 128 : (k + 1) * 128]
    dbd = {
        f"db{c_}": np.ascontiguousarray(dbx[:, :, c_ * GW : (c_ + 1) * GW])
        for c_ in range(1, NCG)
    }
    dbd["db0k0"] = np.ascontiguousarray(dbx[:, 0, 0:GW])
    dbd["db0k1"] = np.ascontiguousarray(dbx[:, 1, 0:GW])
    # -0.5||y||^2 broadcast rows: rolled s1 side then s2 side, fp16
    c1_64 = -0.5 * np.square(s1p.astype(np.float64)).sum(1)
    crow_half = np.concatenate([c1_64.astype(np.float16), c2_half])
    nbc = np.ascontiguousarray(np.broadcast_to(crow_half[None, :], (128, 2 * N1)))
    for c_ in range(NCG):
        dbd[f"nbc{c_}"] = np.ascontiguousarray(nbc[:, c_ * GW : (c_ + 1) * GW])
    return dbd


def kernel(s1, s2, k):
    assert int(k) == 1
    from concourse.bass_utils import run_bass_kernel_spmd
    import ml_dtypes

    s1 = np.asarray(s1, dtype=np.float32)
    s2 = np.asarray(s2, dtype=np.float32)
    s2T_bf = np.ascontiguousarray(s2.T).astype(ml_dtypes.bfloat16)
    c2_64 = -0.5 * np.square(s2.astype(np.float64)).sum(1)
    c2_half = c2_64.astype(np.float16)
    c2_hi, c2_lo = _bf16_hilo(c2_64)
    nrmh = np.ones((2, N1 + 128), dtype=ml_dtypes.bfloat16)
    nrmh[0, 0:N1], nrmh[1, 0:N1] = c2_hi, c2_lo
    diagw = (
        np.arange(RT, dtype=np.float32)[None, :] * 128
        + np.arange(128, dtype=np.float32)[:, None]
    )
    diagw = np.ascontiguousarray(diagw)

    nc = _build_nc()
    in_maps = []
    for c in range(NCORES):
        dbd = _prep_core(s1, s2T_bf, c2_half, c)
        in_maps.append({**dbd, "diagw": diagw, "nrmh": nrmh})
    import os

    res = run_bass_kernel_spmd(
        nc,
        in_maps,
        core_ids=list(range(NCORES)),
        trace=os.environ.get("KBENCH_TRACE") == "1",
    )
    kernel.last_results = res

    # host epilogue (float64): rho/nu from per-group maxes, then the estimator
    sq1 = np.square(s1.astype(np.float64)).sum(1)
    total = 0.0
    for c in range(NCORES):
        part = res.results[c]["o"].astype(np.float64)  # [128, RT, 5]
        maxA = part[:, :, 0:4].max(axis=2)  # [128, RT]
        maxB = part[:, :, 4]
        idx = np.arange(RT)[None, :] * 128 + np.arange(128)[:, None]
        orig = (c * QPC + idx) % N1
        sqx = sq1[orig]
        rho_sq = sqx - 2.0 * maxA
        nu_sq = sqx - 2.0 * maxB
        rho_sq = np.maximum(rho_sq, 1e-20)
        nu_sq = np.maximum(nu_sq, 1e-20)
        total += 0.5 * (np.log(nu_sq) - np.log(rho_sq)).sum()
    base = np.log(N2 / (N1 - 1))
    return np.float32(base + (D / N1) * total)


# revision 19
# speedup vs baseline: 1.2337x; 1.2337x over previous
import sys

sys.path.insert(0, "/opt/trn_rl_repo")
import numpy as np

N1, N2, D = 8192, 8192, 256
NCORES = 8
QPC = N1 // NCORES  # queries per core (1024)
RT = QPC // 128  # row tiles per core (8)
GW = 2048  # colgroup width (4 psum banks)
NCG = 8  # colgroups (4 per side)

# Staged colgroups: PE adds norms via rank-2 matmuls, ACT copies psum->fp16,
# maxes accumulate on DVE via fp16 tensor_tensor (2x mode), one stock reduce
# per query tile. GpSimd must stay idle: its SBUF traffic poisons DVE 2x
# throughput (measured 1134ns -> 4990ns per TT while GpSimd runs).
STAGED = (4, 5, 6)
FIRST_STAGED = 4
CG_ORDER = (0, 4, 1, 5, 2, 6, 3, 7)
# parts column per direct cg; staged side-1 max lands in column 4
PCOL = {0: 0, 1: 1, 2: 2, 3: 3, 7: 5}

_OP_NAME = "TT_ADD_MAX_DIAG_ANT"


def _register_custom_op():
    """Fused DVE op: body = (idx==c0 ? -FLT_MAX : in0 + in1), accum = max."""
    import concourse.dve_ops as dve_ops

    for op in dve_ops.OPS:
        if op.name == _OP_NAME:
            return op
    from concourse.dve_spec import (
        C0,
        Idx,
        MaxNeg,
        Spec,
        Src0,
        Src1,
        eq,
        lower,
        maxx,
        select,
        _has_src1,
    )
    from concourse.dve_uop import DveOpSpec

    def _ref(in0, in1, c0, c1, c2):
        P = in0.shape[0]
        x = in0.astype(np.float32).reshape(P, -1)
        y = np.broadcast_to(np.asarray(in1, np.float32).reshape(P, -1), x.shape)
        n = x.shape[1]
        idx = np.broadcast_to(np.arange(n, dtype=np.float32), (P, n))
        c0b = np.broadcast_to(np.asarray(c0, np.float32).reshape(-1, 1), (P, 1))
        fmin = np.float32(np.finfo(np.float32).min)
        body = np.where(idx == c0b, fmin, x + y).astype(np.float32)
        acc = np.maximum(np.max(body, axis=-1, keepdims=True), fmin)
        return body, acc

    spec = Spec(
        body=select(eq(Idx, C0), MaxNeg, Src0 + Src1),
        accum=maxx,
        reference=_ref,
    )
    row = dve_ops._CUSTOM_DVE_ROW_BASE + len(dve_ops.OPS)
    shas = {}
    for ver in ("v3", "v4"):
        try:
            u = lower(spec, ver=ver)
            shas[ver] = DveOpSpec(
                name=_OP_NAME, opcode=row, uops=u, rd1_en=_has_src1(spec)
            ).sha(ver)
        except ValueError:
            pass
    op = dve_ops.DveOp(_OP_NAME, spec, subdim=False, uops_sha=shas)
    dve_ops.OPS.append(op)
    dve_ops._SUB_OPCODE_FOR_NAME[_OP_NAME] = row
    dve_ops.CUSTOM_DVE_SPECS[_OP_NAME] = spec
    return op


def _build_nc():
    import concourse.bass as bass
    import concourse.tile as tile
    from concourse import mybir

    ttop = _register_custom_op()

    f32 = mybir.dt.float32
    bf16 = mybir.dt.bfloat16
    fp16 = mybir.dt.float16
    max_op = mybir.AluOpType.max
    add_op = mybir.AluOpType.add

    nc = bass.Bass()
    # cg0 split by contraction half so the two DMA queues race it in first
    db0kd = [
        nc.dram_tensor(f"db0k{k}", [128, GW], bf16, kind="ExternalInput")
        for k in (0, 1)
    ]
    dbd = [None] + [
        nc.dram_tensor(f"db{c}", [128, 2, GW], bf16, kind="ExternalInput")
        for c in range(1, NCG)
    ]
    nbcd = [
        nc.dram_tensor(f"nbc{c}", [128, GW], fp16, kind="ExternalInput")
        for c in range(NCG)
    ]
    # bf16 hi/lo rows of side-1 -0.5||y||^2 plus trailing ones block
    nrmhd = nc.dram_tensor("nrmh", [2, N1 + 128], bf16, kind="ExternalInput")
    diagwd = nc.dram_tensor("diagw", [128, RT], f32, kind="ExternalInput")
    o = nc.dram_tensor("o", [128, RT, 6], f32, kind="ExternalOutput")

    with tile.TileContext(nc) as tc:
        with (
            tc.tile_pool(name="sb", bufs=1) as sb,
            tc.tile_pool(name="pp", bufs=1) as pp,
            tc.tile_pool(name="st", bufs=3) as st,
            tc.tile_pool(name="ps", bufs=2, space="PSUM") as ps,
        ):
            # DMA plan: small/critical first; db/nbc chunks land in
            # consumption order, spread over the three DGE queues.
            tdw = sb.tile([128, RT], f32, tag="dw")
            nc.gpsimd.dma_start(out=tdw, in_=diagwd[:])
            tnbc = [
                sb.tile([128, GW], fp16, name=f"nbc{c}", tag=f"nbc{c}")
                for c in range(NCG)
            ]
            nc.gpsimd.dma_start(out=tnbc[0], in_=nbcd[0][:])
            tnh = sb.tile([2, N1 + 128], bf16, tag="nh")
            nc.gpsimd.dma_start(out=tnh, in_=nrmhd[:])
            qs = [nc.sync, nc.scalar]
            tdb0k = [
                sb.tile([128, GW], bf16, name=f"db0k{k}", tag=f"db0k{k}")
                for k in (0, 1)
            ]
            nc.sync.dma_start(out=tdb0k[0], in_=db0kd[0][:])
            nc.scalar.dma_start(out=tdb0k[1], in_=db0kd[1][:])
            tdb = [None] + [
                sb.tile([128, 2, GW], bf16, name=f"db{c}", tag=f"db{c}")
                for c in range(1, NCG)
            ]
            for c in range(1, NCG):
                qs[c % 2].dma_start(out=tdb[c], in_=dbd[c][:])
            for c in range(1, NCG):
                nc.gpsimd.dma_start(out=tnbc[c], in_=nbcd[c][:])

            # wait absorbers: DVE observes the startup-critical DMAs once
            dum = sb.tile([128, 2], f32, tag="dum")
            nc.vector.tensor_copy(out=dum[:, 0:1], in_=tdw[:, 0:1])
            nc.vector.tensor_copy(out=dum[:, 1:2], in_=tnbc[0][:, 0:1])
            dumo = sb.tile([128, 1], f32, tag="dumo")

            ones2 = tnh[0:2, N1 : N1 + 128]
            parts = [
                pp.tile([128, 6], f32, name=f"part{m}", tag=f"part{m}")
                for m in range(RT)
            ]
            accs = [
                pp.tile([128, GW], fp16, name=f"acc{m}", tag=f"acc{m}")
                for m in range(RT)
            ]
            for cg in CG_ORDER:
                for m in range(RT):
                    lhs = [tdb0k[k][:, m * 128 : (m + 1) * 128] for k in (0, 1)]
                    pst = ps.tile([128, GW], f32, tag="pst")
                    pe_norm = cg in STAGED
                    for k in (0, 1):
                        for i in (0, 1, 2, 3):
                            rhs = (
                                tdb0k[k][:, i * 512 : (i + 1) * 512]
                                if cg == 0
                                else tdb[cg][:, k, i * 512 : (i + 1) * 512]
                            )
                            nc.tensor.matmul(
                                out=pst[:, i * 512 : (i + 1) * 512],
                                lhsT=lhs[k],
                                rhs=rhs,
                                start=(k == 0),
                                stop=(k == 1 and not pe_norm),
                            )
                    if pe_norm:
                        nb = (cg - 4) * GW
                        for i in (0, 1, 2, 3):
                            nc.tensor.matmul(
                                out=pst[:, i * 512 : (i + 1) * 512],
                                lhsT=ones2,
                                rhs=tnh[0:2, nb + i * 512 : nb + (i + 1) * 512],
                                start=False,
                                stop=True,
                            )
                    if cg not in STAGED:
                        pc = PCOL[cg]
                        nc.vector._custom_dve(
                            ttop,
                            out=dumo.broadcast_to(pst.shape),
                            in0=pst,
                            in1=tnbc[cg][:, :],
                            s0=tdw[:, m : m + 1] if cg == 0 else -1.0,
                            s1=0.0,
                            imm2=0.0,
                            accum_out=parts[m][:, pc : pc + 1],
                        )
                        continue
                    if cg == FIRST_STAGED:
                        tgt = accs[m]
                    else:
                        tgt = st.tile([128, GW], fp16, tag="stg")
                    nc.scalar.copy(out=tgt, in_=pst)
                    if cg != FIRST_STAGED:
                        nc.vector.tensor_tensor(
                            out=accs[m], in0=accs[m], in1=tgt, op=max_op
                        )
                    if cg == STAGED[-1]:
                        nc.vector.tensor_reduce(
                            out=parts[m][:, 4:5],
                            in_=accs[m],
                            axis=mybir.AxisListType.X,
                            op=max_op,
                        )
            for m in range(RT):
                nc.sync.dma_start(out=o[:, m, :], in_=parts[m])

    from concourse.bass import _bass_rust
    from concourse.library_overlay import lower_extended_insts

    lower_extended_insts(nc)  # populate .instr for InstISA subclasses
    _bass_rust.move_matmul_waits_to_ldweights(nc.m)
    _bass_rust.generate_event_semaphores(nc)
    return nc


def _bf16_hilo(x64):
    import ml_dtypes

    hi = x64.astype(np.float32).astype(ml_dtypes.bfloat16)
    lo = (x64 - hi.astype(np.float64)).astype(np.float32).astype(ml_dtypes.bfloat16)
    return hi, lo


def _prep_core(s1, s2T_bf, c2_half, c):
    import ml_dtypes

    bf = ml_dtypes.bfloat16
    s1p = np.roll(s1, -c * QPC, axis=0)
    s1pT_bf = np.ascontiguousarray(s1p.T).astype(bf)
    dbx = np.empty((128, 2, 2 * N1), dtype=bf)
    for k in (0, 1):
        dbx[:, k, 0:N1] = s1pT_bf[k * 128 : (k + 1) * 128]
        dbx[:, k, N1 : 2 * N1] = s2T_bf[k * 128 : (k + 1) * 128]
    dbd = {
        f"db{c_}": np.ascontiguousarray(dbx[:, :, c_ * GW : (c_ + 1) * GW])
        for c_ in range(1, NCG)
    }
    dbd["db0k0"] = np.ascontiguousarray(dbx[:, 0, 0:GW])
    dbd["db0k1"] = np.ascontiguousarray(dbx[:, 1, 0:GW])
    # -0.5||y||^2 broadcast rows: rolled s1 side then s2 side, fp16
    c1_64 = -0.5 * np.square(s1p.astype(np.float64)).sum(1)
    crow_half = np.concatenate([c1_64.astype(np.float16), c2_half])
    nbc = np.ascontiguousarray(np.broadcast_to(crow_half[None, :], (128, 2 * N1)))
    for c_ in range(NCG):
        dbd[f"nbc{c_}"] = np.ascontiguousarray(nbc[:, c_ * GW : (c_ + 1) * GW])
    return dbd


def kernel(s1, s2, k):
    assert int(k) == 1
    from concourse.bass_utils import run_bass_kernel_spmd
    import ml_dtypes

    s1 = np.asarray(s1, dtype=np.float32)
    s2 = np.asarray(s2, dtype=np.float32)
    s2T_bf = np.ascontiguousarray(s2.T).astype(ml_dtypes.bfloat16)
    c2_64 = -0.5 * np.square(s2.astype(np.float64)).sum(1)
    c2_half = c2_64.astype(np.float16)
    c2_hi, c2_lo = _bf16_hilo(c2_64)
    nrmh = np.ones((2, N1 + 128), dtype=ml_dtypes.bfloat16)
    nrmh[0, 0:N1], nrmh[1, 0:N1] = c2_hi, c2_lo
    diagw = (
        np.arange(RT, dtype=np.float32)[None, :] * 128
        + np.arange(128, dtype=np.float32)[:, None]
    )
    diagw = np.ascontiguousarray(diagw)

    nc = _build_nc()
    in_maps = []
    for c in range(NCORES):
        dbd = _prep_core(s1, s2T_bf, c2_half, c)
        in_maps.append({**dbd, "diagw": diagw, "nrmh": nrmh})
    import os

    res = run_bass_kernel_spmd(
        nc,
        in_maps,
        core_ids=list(range(NCORES)),
        trace=os.environ.get("KBENCH_TRACE") == "1",
    )
    kernel.last_results = res

    # host epilogue (float64): rho/nu from per-group maxes, then the estimator
    sq1 = np.square(s1.astype(np.float64)).sum(1)
    total = 0.0
    for c in range(NCORES):
        part = res.results[c]["o"].astype(np.float64)  # [128, RT, 6]
        maxA = part[:, :, 0:4].max(axis=2)  # [128, RT]
        maxB = part[:, :, 4:6].max(axis=2)
        idx = np.arange(RT)[None, :] * 128 + np.arange(128)[:, None]
        orig = (c * QPC + idx) % N1
        sqx = sq1[orig]
        rho_sq = sqx - 2.0 * maxA
        nu_sq = sqx - 2.0 * maxB
        rho_sq = np.maximum(rho_sq, 1e-20)
        nu_sq = np.maximum(nu_sq, 1e-20)
        total += 0.5 * (np.log(nu_sq) - np.log(rho_sq)).sum()
    base = np.log(N2 / (N1 - 1))
    return np.float32(base + (D / N1) * total)


# revision 24
# speedup vs baseline: 1.2562x; 1.0182x over previous
import sys

sys.path.insert(0, "/opt/trn_rl_repo")
import numpy as np

N1, N2, D = 8192, 8192, 256
NCORES = 8
QPC = N1 // NCORES  # queries per core (1024)
RT = QPC // 128  # row tiles per core (8)
GW = 2048  # colgroup width (4 psum banks)
NCG = 8  # colgroups (4 per side)

# cg roles:
#   0,1,2,3,6 direct: fused DVE op adds the broadcast norm row and
#     max-reduces straight from PSUM (cg0 with the self-exclusion window).
#   4,5 staged: PE adds norms (rank-2 matmuls), ACT copies psum->fp16,
#     DVE accumulates via fp16 tensor_tensor max (2x mode).
#   7 merge: PE adds norms; a second fused op max-reduces over
#     max(psum_cg7, acc) so side 1 needs no stock reduce at all.
# GpSimd must stay idle: its SBUF traffic poisons DVE 2x throughput.
DIRECT = (0, 1, 2, 3, 6)
PCOL = {0: 0, 1: 1, 2: 2, 3: 3, 6: 5}  # parts column per direct cg
NORMED = (4, 5, 7)  # cgs whose norms come from PE rank-2 matmuls

_OP1 = "TT_ADD_MAX_DIAG_ANT"
_OP2 = "TT_MAXMERGE_MAX_ANT"


def _register_custom_ops():
    """Two fused DVE ops, registered via the documented dve_ops extension
    point (append to OPS):
      op1: body = (idx==c0 ? -FLT_MAX : in0 + in1), accum = max
      op2: body = max(in0, in1), accum = max
    """
    import concourse.dve_ops as dve_ops

    have = {op.name: op for op in dve_ops.OPS}
    if _OP1 in have and _OP2 in have:
        return have[_OP1], have[_OP2]
    from concourse.dve_spec import (
        C0,
        Idx,
        MaxNeg,
        Spec,
        Src0,
        Src1,
        eq,
        lower,
        maxx,
        select,
        _has_src1,
    )
    from concourse.dve_uop import DveOpSpec

    def _ref1(in0, in1, c0, c1, c2):
        P = in0.shape[0]
        x = in0.astype(np.float32).reshape(P, -1)
        y = np.broadcast_to(np.asarray(in1, np.float32).reshape(P, -1), x.shape)
        n = x.shape[1]
        idx = np.broadcast_to(np.arange(n, dtype=np.float32), (P, n))
        c0b = np.broadcast_to(np.asarray(c0, np.float32).reshape(-1, 1), (P, 1))
        fmin = np.float32(np.finfo(np.float32).min)
        body = np.where(idx == c0b, fmin, x + y).astype(np.float32)
        return body, np.max(body, axis=-1, keepdims=True)

    def _ref2(in0, in1, c0, c1, c2):
        P = in0.shape[0]
        x = in0.astype(np.float32).reshape(P, -1)
        y = np.broadcast_to(np.asarray(in1, np.float32).reshape(P, -1), x.shape)
        body = np.maximum(x, y).astype(np.float32)
        return body, np.max(body, axis=-1, keepdims=True)

    specs = {
        _OP1: Spec(
            body=select(eq(Idx, C0), MaxNeg, Src0 + Src1), accum=maxx, reference=_ref1
        ),
        _OP2: Spec(body=maxx(Src0, Src1), accum=maxx, reference=_ref2),
    }
    out = []
    for name, spec in specs.items():
        if name in have:
            out.append(have[name])
            continue
        row = dve_ops._CUSTOM_DVE_ROW_BASE + len(dve_ops.OPS)
        shas = {}
        for ver in ("v3", "v4"):
            try:
                u = lower(spec, ver=ver)
                shas[ver] = DveOpSpec(
                    name=name, opcode=row, uops=u, rd1_en=_has_src1(spec)
                ).sha(ver)
            except ValueError:
                pass
        op = dve_ops.DveOp(name, spec, subdim=False, uops_sha=shas)
        dve_ops.OPS.append(op)
        dve_ops._SUB_OPCODE_FOR_NAME[name] = row
        dve_ops.CUSTOM_DVE_SPECS[name] = spec
        out.append(op)
    return out


def _build_nc():
    import concourse.bass as bass
    import concourse.tile as tile
    from concourse import mybir

    op1, op2 = _register_custom_ops()

    f32 = mybir.dt.float32
    bf16 = mybir.dt.bfloat16
    fp16 = mybir.dt.float16
    max_op = mybir.AluOpType.max

    nc = bass.Bass()
    # cg0 split by contraction half and column half so the two DMA queues
    # race the opening tiles in
    db0kd = [
        nc.dram_tensor(f"db0k{k}", [128, GW], bf16, kind="ExternalInput")
        for k in (0, 1)
    ]
    db4kd = [
        nc.dram_tensor(f"db4k{k}", [128, GW], bf16, kind="ExternalInput")
        for k in (0, 1)
    ]
    dbd = [None] + [
        nc.dram_tensor(f"db{c}", [128, 2, GW], bf16, kind="ExternalInput")
        for c in range(1, NCG)
        if c != 4
    ]
    dbd.insert(4, None)
    nbcd = {
        c: nc.dram_tensor(f"nbc{c}", [128, GW], fp16, kind="ExternalInput")
        for c in DIRECT
    }
    # bf16 hi/lo rows of side-1 -0.5||y||^2 plus trailing ones block
    nrmhd = nc.dram_tensor("nrmh", [2, N1 + 128], bf16, kind="ExternalInput")
    diagwd = nc.dram_tensor("diagw", [128, RT], f32, kind="ExternalInput")
    o = nc.dram_tensor("o", [128, RT, 6], f32, kind="ExternalOutput")

    with tile.TileContext(nc) as tc:
        with (
            tc.tile_pool(name="sb", bufs=1) as sb,
            tc.tile_pool(name="pp", bufs=1) as pp,
            tc.tile_pool(name="st", bufs=3) as st,
            tc.tile_pool(name="ps", bufs=2, space="PSUM") as ps,
        ):
            # DMA plan: small/critical first; db chunks land in consumption
            # order (phase A: cg0+cg4; phase B: 1,5,2,6,3,7)
            tdw = sb.tile([128, RT], f32, tag="dw")
            nc.gpsimd.dma_start(out=tdw, in_=diagwd[:])
            tnbc = {}
            for c in DIRECT:
                tnbc[c] = sb.tile([128, GW], fp16, name=f"nbc{c}", tag=f"nbc{c}")
            nc.gpsimd.dma_start(out=tnbc[0], in_=nbcd[0][:])
            tnh = sb.tile([2, N1 + 128], bf16, tag="nh")
            nc.gpsimd.dma_start(out=tnh, in_=nrmhd[:])
            tdb0k = [
                sb.tile([128, GW], bf16, name=f"db0k{k}", tag=f"db0k{k}")
                for k in (0, 1)
            ]
            nc.sync.dma_start(out=tdb0k[0], in_=db0kd[0][:])
            nc.scalar.dma_start(out=tdb0k[1], in_=db0kd[1][:])
            tdb4k = [
                sb.tile([128, GW], bf16, name=f"db4k{k}", tag=f"db4k{k}")
                for k in (0, 1)
            ]
            nc.sync.dma_start(out=tdb4k[0], in_=db4kd[0][:])
            nc.scalar.dma_start(out=tdb4k[1], in_=db4kd[1][:])
            tdb = [
                sb.tile([128, 2, GW], bf16, name=f"db{c}", tag=f"db{c}")
                if c not in (0, 4)
                else None
                for c in range(NCG)
            ]
            for i, c in enumerate((1, 5, 2, 6, 3, 7)):
                (nc.scalar if i % 2 == 0 else nc.sync).dma_start(
                    out=tdb[c], in_=dbd[c][:]
                )
            for c in (1, 2, 6, 3):
                nc.gpsimd.dma_start(out=tnbc[c], in_=nbcd[c][:])

            # wait absorbers: DVE observes the startup-critical DMAs once
            dum = sb.tile([128, 2], f32, tag="dum")
            nc.vector.tensor_copy(out=dum[:, 0:1], in_=tdw[:, 0:1])
            nc.vector.tensor_copy(out=dum[:, 1:2], in_=tnbc[0][:, 0:1])
            dumo = sb.tile([128, 1], f32, tag="dumo")

            ones2 = tnh[0:2, N1 : N1 + 128]
            parts = [
                pp.tile([128, 6], f32, name=f"part{m}", tag=f"part{m}")
                for m in range(RT)
            ]
            accs = [
                pp.tile([128, GW], fp16, name=f"acc{m}", tag=f"acc{m}")
                for m in range(RT)
            ]

            def tile_for(cg, m):
                lhs = [tdb0k[k][:, m * 128 : (m + 1) * 128] for k in (0, 1)]
                pst = ps.tile([128, GW], f32, tag="pst")
                pe_norm = cg in NORMED
                for k in (0, 1):
                    for i in (0, 1, 2, 3):
                        if cg == 0:
                            rhs = tdb0k[k][:, i * 512 : (i + 1) * 512]
                        elif cg == 4:
                            rhs = tdb4k[k][:, i * 512 : (i + 1) * 512]
                        else:
                            rhs = tdb[cg][:, k, i * 512 : (i + 1) * 512]
                        nc.tensor.matmul(
                            out=pst[:, i * 512 : (i + 1) * 512],
                            lhsT=lhs[k],
                            rhs=rhs,
                            start=(k == 0),
                            stop=(k == 1 and not pe_norm),
                        )
                if pe_norm:
                    nb = (cg - 4) * GW
                    for i in (0, 1, 2, 3):
                        nc.tensor.matmul(
                            out=pst[:, i * 512 : (i + 1) * 512],
                            lhsT=ones2,
                            rhs=tnh[0:2, nb + i * 512 : nb + (i + 1) * 512],
                            start=False,
                            stop=True,
                        )
                if cg in DIRECT:
                    pc = PCOL[cg]
                    nc.vector._custom_dve(
                        op1,
                        out=dumo.broadcast_to(pst.shape),
                        in0=pst,
                        in1=tnbc[cg][:, :],
                        s0=tdw[:, m : m + 1] if cg == 0 else -1.0,
                        s1=0.0,
                        imm2=0.0,
                        accum_out=parts[m][:, pc : pc + 1],
                    )
                elif cg == 4:
                    nc.scalar.copy(out=accs[m], in_=pst)
                elif cg == 5:
                    stg = st.tile([128, GW], fp16, tag="stg")
                    nc.scalar.copy(out=stg, in_=pst)
                    nc.vector.tensor_tensor(
                        out=accs[m], in0=accs[m], in1=stg, op=max_op
                    )
                else:  # cg == 7: merge side-1 accumulator into this scan
                    nc.vector._custom_dve(
                        op2,
                        out=dumo.broadcast_to(pst.shape),
                        in0=pst,
                        in1=accs[m],
                        s0=0.0,
                        s1=0.0,
                        imm2=0.0,
                        accum_out=parts[m][:, 4:5],
                    )
                    nc.sync.dma_start(out=o[:, m, :], in_=parts[m])

            # phase A: cg0 + cg4 while the rest of the database streams in
            for m in range(RT):
                tile_for(0, m)
                tile_for(4, m)
            # phase B: full tile-level interleave — every window carries the
            # average engine mix
            for m in range(RT):
                for cg in (1, 5, 2, 6, 3, 7):
                    tile_for(cg, m)

    from concourse.bass import _bass_rust
    from concourse.library_overlay import lower_extended_insts

    lower_extended_insts(nc)  # populate .instr for InstISA subclasses
    _bass_rust.move_matmul_waits_to_ldweights(nc.m)
    _bass_rust.generate_event_semaphores(nc)
    return nc


def _bf16_hilo(x64):
    import ml_dtypes

    hi = x64.astype(np.float32).astype(ml_dtypes.bfloat16)
    lo = (x64 - hi.astype(np.float64)).astype(np.float32).astype(ml_dtypes.bfloat16)
    return hi, lo


def _prep_core(s1, s2T_bf, c2_half, c):
    import ml_dtypes

    bf = ml_dtypes.bfloat16
    s1p = np.roll(s1, -c * QPC, axis=0)
    s1pT_bf = np.ascontiguousarray(s1p.T).astype(bf)
    dbx = np.empty((128, 2, 2 * N1), dtype=bf)
    for k in (0, 1):
        dbx[:, k, 0:N1] = s1pT_bf[k * 128 : (k + 1) * 128]
        dbx[:, k, N1 : 2 * N1] = s2T_bf[k * 128 : (k + 1) * 128]
    dbd = {
        f"db{c_}": np.ascontiguousarray(dbx[:, :, c_ * GW : (c_ + 1) * GW])
        for c_ in range(1, NCG)
        if c_ != 4
    }
    dbd["db0k0"] = np.ascontiguousarray(dbx[:, 0, 0:GW])
    dbd["db0k1"] = np.ascontiguousarray(dbx[:, 1, 0:GW])
    dbd["db4k0"] = np.ascontiguousarray(dbx[:, 0, 4 * GW : 5 * GW])
    dbd["db4k1"] = np.ascontiguousarray(dbx[:, 1, 4 * GW : 5 * GW])
    # -0.5||y||^2 broadcast rows for the direct colgroups, fp16
    c1_64 = -0.5 * np.square(s1p.astype(np.float64)).sum(1)
    crow_half = np.concatenate([c1_64.astype(np.float16), c2_half])
    nbc = np.ascontiguousarray(np.broadcast_to(crow_half[None, :], (128, 2 * N1)))
    for c_ in DIRECT:
        dbd[f"nbc{c_}"] = np.ascontiguousarray(nbc[:, c_ * GW : (c_ + 1) * GW])
    return dbd


def kernel(s1, s2, k):
    assert int(k) == 1
    from concourse.bass_utils import run_bass_kernel_spmd
    import ml_dtypes

    s1 = np.asarray(s1, dtype=np.float32)
    s2 = np.asarray(s2, dtype=np.float32)
    s2T_bf = np.ascontiguousarray(s2.T).astype(ml_dtypes.bfloat16)
    c2_64 = -0.5 * np.square(s2.astype(np.float64)).sum(1)
    c2_half = c2_64.astype(np.float16)
    c2_hi, c2_lo = _bf16_hilo(c2_64)
    nrmh = np.ones((2, N1 + 128), dtype=ml_dtypes.bfloat16)
    nrmh[0, 0:N1], nrmh[1, 0:N1] = c2_hi, c2_lo
    diagw = (
        np.arange(RT, dtype=np.float32)[None, :] * 128
        + np.arange(128, dtype=np.float32)[:, None]
    )
    diagw = np.ascontiguousarray(diagw)

    nc = _build_nc()
    in_maps = []
    for c in range(NCORES):
        dbd = _prep_core(s1, s2T_bf, c2_half, c)
        in_maps.append({**dbd, "diagw": diagw, "nrmh": nrmh})
    import os

    res = run_bass_kernel_spmd(
        nc,
        in_maps,
        core_ids=list(range(NCORES)),
        trace=os.environ.get("KBENCH_TRACE") == "1",
    )
    kernel.last_results = res

    # host epilogue (float64): rho/nu from per-group maxes, then the estimator
    sq1 = np.square(s1.astype(np.float64)).sum(1)
    total = 0.0
    for c in range(NCORES):
        part = res.results[c]["o"].astype(np.float64)  # [128, RT, 6]
        maxA = part[:, :, 0:4].max(axis=2)  # [128, RT]
        maxB = part[:, :, 4:6].max(axis=2)
        idx = np.arange(RT)[None, :] * 128 + np.arange(128)[:, None]
        orig = (c * QPC + idx) % N1
        sqx = sq1[orig]
        rho_sq = sqx - 2.0 * maxA
        nu_sq = sqx - 2.0 * maxB
        rho_sq = np.maximum(rho_sq, 1e-20)
        nu_sq = np.maximum(nu_sq, 1e-20)
        total += 0.5 * (np.log(nu_sq) - np.log(rho_sq)).sum()
    base = np.log(N2 / (N1 - 1))
    return np.float32(base + (D / N1) * total)


# revision 33
# speedup vs baseline: 1.2899x; 1.0268x over previous
import sys

sys.path.insert(0, "/opt/trn_rl_repo")
import numpy as np

N1, N2, D = 8192, 8192, 256
NCORES = 8
QPC = N1 // NCORES  # queries per core (1024)
RT = QPC // 128  # row tiles per core (8)
GW = 2048  # colgroup width (4 psum banks)
NCG = 8  # colgroups (4 per side)

# Tile roles (PE runs ~2.0GHz sustained, so only ~8 tiles of norm matmuls
# can move to PE before it becomes the bottleneck):
#   most tiles direct: fused DVE op adds the broadcast norm row and
#     max-reduces straight from PSUM (cg0 with the self-exclusion window).
#   cg4 with m<HM: PE adds norms (rank-2 matmuls), ACT copies psum->fp16 acc.
#   cg7 with m<HM: PE adds norms; second fused op max-reduces over
#     max(psum_cg7, acc) — no stock reduce needed.
# GpSimd must stay idle: its SBUF traffic poisons DVE 2x throughput.
HM = 4  # m-range of the hybrid (staged/merged) route for cg4/cg7
PCOL = {0: 0, 1: 1, 2: 2, 3: 3, 4: 4, 5: 5, 6: 6, 7: 7}

_OP1 = "TT_ADD_MAX_DIAG_ANT"
_OP2 = "TT_MAXMERGE_MAX_ANT"


def _register_custom_ops():
    """Two fused DVE ops, registered via the documented dve_ops extension
    point (append to OPS):
      op1: body = (idx==c0 ? -FLT_MAX : in0 + in1), accum = max
      op2: body = max(in0, in1), accum = max
    """
    import concourse.dve_ops as dve_ops

    have = {op.name: op for op in dve_ops.OPS}
    if _OP1 in have and _OP2 in have:
        return have[_OP1], have[_OP2]
    from concourse.dve_spec import (
        C0,
        Idx,
        MaxNeg,
        Spec,
        Src0,
        Src1,
        eq,
        lower,
        maxx,
        select,
        _has_src1,
    )
    from concourse.dve_uop import DveOpSpec

    def _ref1(in0, in1, c0, c1, c2):
        P = in0.shape[0]
        x = in0.astype(np.float32).reshape(P, -1)
        y = np.broadcast_to(np.asarray(in1, np.float32).reshape(P, -1), x.shape)
        n = x.shape[1]
        idx = np.broadcast_to(np.arange(n, dtype=np.float32), (P, n))
        c0b = np.broadcast_to(np.asarray(c0, np.float32).reshape(-1, 1), (P, 1))
        fmin = np.float32(np.finfo(np.float32).min)
        body = np.where(idx == c0b, fmin, x + y).astype(np.float32)
        return body, np.max(body, axis=-1, keepdims=True)

    def _ref2(in0, in1, c0, c1, c2):
        P = in0.shape[0]
        x = in0.astype(np.float32).reshape(P, -1)
        y = np.broadcast_to(np.asarray(in1, np.float32).reshape(P, -1), x.shape)
        body = np.maximum(x, y).astype(np.float32)
        return body, np.max(body, axis=-1, keepdims=True)

    specs = {
        _OP1: Spec(
            body=select(eq(Idx, C0), MaxNeg, Src0 + Src1), accum=maxx, reference=_ref1
        ),
        _OP2: Spec(body=maxx(Src0, Src1), accum=maxx, reference=_ref2),
    }
    out = []
    for name, spec in specs.items():
        if name in have:
            out.append(have[name])
            continue
        row = dve_ops._CUSTOM_DVE_ROW_BASE + len(dve_ops.OPS)
        shas = {}
        for ver in ("v3", "v4"):
            try:
                u = lower(spec, ver=ver)
                shas[ver] = DveOpSpec(
                    name=name, opcode=row, uops=u, rd1_en=_has_src1(spec)
                ).sha(ver)
            except ValueError:
                pass
        op = dve_ops.DveOp(name, spec, subdim=False, uops_sha=shas)
        dve_ops.OPS.append(op)
        dve_ops._SUB_OPCODE_FOR_NAME[name] = row
        dve_ops.CUSTOM_DVE_SPECS[name] = spec
        out.append(op)
    return out


def _build_nc():
    import concourse.bass as bass
    import concourse.tile as tile
    from concourse import mybir

    op1, op2 = _register_custom_ops()

    f32 = mybir.dt.float32
    bf16 = mybir.dt.bfloat16
    fp16 = mybir.dt.float16
    max_op = mybir.AluOpType.max

    nc = bass.Bass()
    # cg0 split by contraction half and column half so the two DMA queues
    # race the opening tiles in
    db0kd = [
        nc.dram_tensor(f"db0k{k}", [128, GW], bf16, kind="ExternalInput")
        for k in (0, 1)
    ]
    db4kd = [
        nc.dram_tensor(f"db4k{k}", [128, GW], bf16, kind="ExternalInput")
        for k in (0, 1)
    ]
    dbd = [None] + [
        nc.dram_tensor(f"db{c}", [128, 2, GW], bf16, kind="ExternalInput")
        for c in range(1, NCG)
        if c != 4
    ]
    dbd.insert(4, None)
    nbcd = {
        c: nc.dram_tensor(f"nbc{c}", [128, GW], fp16, kind="ExternalInput")
        for c in range(NCG)
    }
    # bf16 hi/lo rows of side-1 -0.5||y||^2 plus trailing ones block
    nrmhd = nc.dram_tensor("nrmh", [2, N1 + 128], bf16, kind="ExternalInput")
    diagwd = nc.dram_tensor("diagw", [128, RT], f32, kind="ExternalInput")
    o = nc.dram_tensor("o", [128, RT, NCG], f32, kind="ExternalOutput")

    with tile.TileContext(nc) as tc:
        with (
            tc.tile_pool(name="sb", bufs=1) as sb,
            tc.tile_pool(name="pp", bufs=1) as pp,
            tc.tile_pool(name="st", bufs=3) as st,
            tc.tile_pool(name="ps", bufs=2, space="PSUM") as ps,
        ):
            # DMA plan: small/critical first; db chunks land in consumption
            # order (phase A: cg0+cg4; phase B: 1,5,2,6,3,7)
            tdw = sb.tile([128, RT], f32, tag="dw")
            nc.gpsimd.dma_start(out=tdw, in_=diagwd[:])
            tnbc = {}
            for c in range(NCG):
                tnbc[c] = sb.tile([128, GW], fp16, name=f"nbc{c}", tag=f"nbc{c}")
            nc.gpsimd.dma_start(out=tnbc[0], in_=nbcd[0][:])
            tnh = sb.tile([2, N1 + 128], bf16, tag="nh")
            nc.gpsimd.dma_start(out=tnh, in_=nrmhd[:])
            tdb0k = [
                sb.tile([128, GW], bf16, name=f"db0k{k}", tag=f"db0k{k}")
                for k in (0, 1)
            ]
            nc.sync.dma_start(out=tdb0k[0], in_=db0kd[0][:])
            nc.scalar.dma_start(out=tdb0k[1], in_=db0kd[1][:])
            tdb4k = [
                sb.tile([128, GW], bf16, name=f"db4k{k}", tag=f"db4k{k}")
                for k in (0, 1)
            ]
            nc.sync.dma_start(out=tdb4k[0], in_=db4kd[0][:])
            nc.scalar.dma_start(out=tdb4k[1], in_=db4kd[1][:])
            tdb = [
                sb.tile([128, 2, GW], bf16, name=f"db{c}", tag=f"db{c}")
                if c not in (0, 4)
                else None
                for c in range(NCG)
            ]
            for i, c in enumerate((1, 5, 2, 6, 3, 7)):
                (nc.scalar if i % 2 == 0 else nc.sync).dma_start(
                    out=tdb[c], in_=dbd[c][:]
                )
            for c in (4, 1, 5, 2, 6, 3, 7):
                nc.gpsimd.dma_start(out=tnbc[c], in_=nbcd[c][:])

            # wait absorbers: DVE observes the startup-critical DMAs once
            dum = sb.tile([128, 2], f32, tag="dum")
            nc.vector.tensor_copy(out=dum[:, 0:1], in_=tdw[:, 0:1])
            nc.vector.tensor_copy(out=dum[:, 1:2], in_=tnbc[0][:, 0:1])
            dumo = sb.tile([128, 1], f32, tag="dumo")

            ones2 = tnh[0:2, N1 : N1 + 128]
            parts = [
                pp.tile([128, NCG], f32, name=f"part{m}", tag=f"part{m}")
                for m in range(RT)
            ]
            accs = [
                pp.tile([128, GW], fp16, name=f"acc{m}", tag=f"acc{m}")
                for m in range(HM)
            ]

            def tile_for(cg, m):
                lhs = [tdb0k[k][:, m * 128 : (m + 1) * 128] for k in (0, 1)]
                pst = ps.tile([128, GW], f32, tag="pst")
                hybrid = cg in (4, 7) and m < HM
                pe_norm = hybrid
                for k in (0, 1):
                    for i in (0, 1, 2, 3):
                        if cg == 0:
                            rhs = tdb0k[k][:, i * 512 : (i + 1) * 512]
                        elif cg == 4:
                            rhs = tdb4k[k][:, i * 512 : (i + 1) * 512]
                        else:
                            rhs = tdb[cg][:, k, i * 512 : (i + 1) * 512]
                        nc.tensor.matmul(
                            out=pst[:, i * 512 : (i + 1) * 512],
                            lhsT=lhs[k],
                            rhs=rhs,
                            start=(k == 0),
                            stop=(k == 1 and not pe_norm),
                        )
                if pe_norm:
                    nb = (cg - 4) * GW
                    for i in (0, 1, 2, 3):
                        nc.tensor.matmul(
                            out=pst[:, i * 512 : (i + 1) * 512],
                            lhsT=ones2,
                            rhs=tnh[0:2, nb + i * 512 : nb + (i + 1) * 512],
                            start=False,
                            stop=True,
                        )
                if not hybrid:
                    pc = PCOL[cg]
                    nc.vector._custom_dve(
                        op1,
                        out=dumo.broadcast_to(pst.shape),
                        in0=pst,
                        in1=tnbc[cg][:, :],
                        s0=tdw[:, m : m + 1] if cg == 0 else -1.0,
                        s1=0.0,
                        imm2=0.0,
                        accum_out=parts[m][:, pc : pc + 1],
                    )
                elif cg == 4:
                    nc.scalar.copy(out=accs[m], in_=pst)
                else:  # cg == 7, m < HM: merge the side-1 accumulator
                    nc.vector._custom_dve(
                        op2,
                        out=dumo.broadcast_to(pst.shape),
                        in0=pst,
                        in1=accs[m],
                        s0=0.0,
                        s1=0.0,
                        imm2=0.0,
                        accum_out=parts[m][:, 4:5],
                    )
                if cg == 7:
                    nc.sync.dma_start(out=o[:, m, :], in_=parts[m])

            # phase A: cg0 + cg4 while the rest of the database streams in
            for m in range(RT):
                tile_for(0, m)
                tile_for(4, m)
            # phase B: full tile-level interleave — every window carries the
            # average engine mix
            for m in range(RT):
                for cg in (1, 5, 2, 6, 3, 7):
                    tile_for(cg, m)

    from concourse.bass import _bass_rust
    from concourse.library_overlay import lower_extended_insts

    lower_extended_insts(nc)  # populate .instr for InstISA subclasses
    _bass_rust.move_matmul_waits_to_ldweights(nc.m)
    _bass_rust.generate_event_semaphores(nc)
    return nc


def _bf16_hilo(x64):
    import ml_dtypes

    hi = x64.astype(np.float32).astype(ml_dtypes.bfloat16)
    lo = (x64 - hi.astype(np.float64)).astype(np.float32).astype(ml_dtypes.bfloat16)
    return hi, lo


def _prep_core(s1, s2T_bf, c2_half, c):
    import ml_dtypes

    bf = ml_dtypes.bfloat16
    s1p = np.roll(s1, -c * QPC, axis=0)
    s1pT_bf = np.ascontiguousarray(s1p.T).astype(bf)
    dbx = np.empty((128, 2, 2 * N1), dtype=bf)
    for k in (0, 1):
        dbx[:, k, 0:N1] = s1pT_bf[k * 128 : (k + 1) * 128]
        dbx[:, k, N1 : 2 * N1] = s2T_bf[k * 128 : (k + 1) * 128]
    dbd = {
        f"db{c_}": np.ascontiguousarray(dbx[:, :, c_ * GW : (c_ + 1) * GW])
        for c_ in range(1, NCG)
        if c_ != 4
    }
    dbd["db0k0"] = np.ascontiguousarray(dbx[:, 0, 0:GW])
    dbd["db0k1"] = np.ascontiguousarray(dbx[:, 1, 0:GW])
    dbd["db4k0"] = np.ascontiguousarray(dbx[:, 0, 4 * GW : 5 * GW])
    dbd["db4k1"] = np.ascontiguousarray(dbx[:, 1, 4 * GW : 5 * GW])
    # -0.5||y||^2 broadcast rows for the direct colgroups, fp16
    c1_64 = -0.5 * np.square(s1p.astype(np.float64)).sum(1)
    crow_half = np.concatenate([c1_64.astype(np.float16), c2_half])
    nbc = np.ascontiguousarray(np.broadcast_to(crow_half[None, :], (128, 2 * N1)))
    for c_ in range(NCG):
        dbd[f"nbc{c_}"] = np.ascontiguousarray(nbc[:, c_ * GW : (c_ + 1) * GW])
    return dbd


def kernel(s1, s2, k):
    assert int(k) == 1
    from concourse.bass_utils import run_bass_kernel_spmd
    import ml_dtypes

    s1 = np.asarray(s1, dtype=np.float32)
    s2 = np.asarray(s2, dtype=np.float32)
    s2T_bf = np.ascontiguousarray(s2.T).astype(ml_dtypes.bfloat16)
    c2_64 = -0.5 * np.square(s2.astype(np.float64)).sum(1)
    c2_half = c2_64.astype(np.float16)
    c2_hi, c2_lo = _bf16_hilo(c2_64)
    nrmh = np.ones((2, N1 + 128), dtype=ml_dtypes.bfloat16)
    nrmh[0, 0:N1], nrmh[1, 0:N1] = c2_hi, c2_lo
    diagw = (
        np.arange(RT, dtype=np.float32)[None, :] * 128
        + np.arange(128, dtype=np.float32)[:, None]
    )
    diagw = np.ascontiguousarray(diagw)

    nc = _build_nc()
    in_maps = []
    for c in range(NCORES):
        dbd = _prep_core(s1, s2T_bf, c2_half, c)
        in_maps.append({**dbd, "diagw": diagw, "nrmh": nrmh})
    import os

    res = run_bass_kernel_spmd(
        nc,
        in_maps,
        core_ids=list(range(NCORES)),
        trace=os.environ.get("KBENCH_TRACE") == "1",
    )
    kernel.last_results = res

    # host epilogue (float64): rho/nu from per-group maxes, then the estimator
    sq1 = np.square(s1.astype(np.float64)).sum(1)
    total = 0.0
    for c in range(NCORES):
        part = res.results[c]["o"].astype(np.float64)  # [128, RT, 8]
        maxA = part[:, :, 0:4].max(axis=2)  # [128, RT]
        # hybrid m<HM: col 4 = merged side-1 max, col 7 unwritten
        maxB = part[:, :, 4:7].max(axis=2)
        maxB[:, HM:] = np.maximum(maxB[:, HM:], part[:, HM:, 7])
        idx = np.arange(RT)[None, :] * 128 + np.arange(128)[:, None]
        orig = (c * QPC + idx) % N1
        sqx = sq1[orig]
        rho_sq = sqx - 2.0 * maxA
        nu_sq = sqx - 2.0 * maxB
        rho_sq = np.maximum(rho_sq, 1e-20)
        nu_sq = np.maximum(nu_sq, 1e-20)
        total += 0.5 * (np.log(nu_sq) - np.log(rho_sq)).sum()
    base = np.log(N2 / (N1 - 1))
    return np.float32(base + (D / N1) * total)


# revision 38
# speedup vs baseline: 1.3353x; 1.0352x over previous
import sys

sys.path.insert(0, "/opt/trn_rl_repo")
import numpy as np

N1, N2, D = 8192, 8192, 256
NCORES = 8
QPC = N1 // NCORES  # queries per core (1024)
RT = QPC // 128  # row tiles per core (8)
GW = 2048  # colgroup width (4 psum banks)
NCG = 8  # colgroups (4 per side)

_OP_NAME = "TT_ADD_MAX_DIAG_ANT"


def _register_custom_op():
    """Fused DVE op: body = (idx==c0 ? -FLT_MAX : in0 + in1), accum = max.

    in0 = psum dot tile, in1 = broadcast -0.5||y||^2 row, c0 = per-partition
    self-column index (or -1 to disable masking). Registered at runtime via
    the documented dve_ops extension point (append to OPS)."""
    import concourse.dve_ops as dve_ops

    for op in dve_ops.OPS:
        if op.name == _OP_NAME:
            return op
    from concourse.dve_spec import (
        C0,
        Idx,
        MaxNeg,
        Spec,
        Src0,
        Src1,
        eq,
        lower,
        maxx,
        select,
        _has_src1,
    )
    from concourse.dve_uop import DveOpSpec

    def _ref(in0, in1, c0, c1, c2):
        P = in0.shape[0]
        x = in0.astype(np.float32).reshape(P, -1)
        y = np.broadcast_to(np.asarray(in1, np.float32).reshape(P, -1), x.shape)
        n = x.shape[1]
        idx = np.broadcast_to(np.arange(n, dtype=np.float32), (P, n))
        c0b = np.broadcast_to(np.asarray(c0, np.float32).reshape(-1, 1), (P, 1))
        fmin = np.float32(np.finfo(np.float32).min)
        body = np.where(idx == c0b, fmin, x + y).astype(np.float32)
        acc = np.maximum(np.max(body, axis=-1, keepdims=True), fmin)
        return body, acc

    spec = Spec(
        body=select(eq(Idx, C0), MaxNeg, Src0 + Src1),
        accum=maxx,
        reference=_ref,
    )
    row = dve_ops._CUSTOM_DVE_ROW_BASE + len(dve_ops.OPS)
    shas = {}
    for ver in ("v3", "v4"):
        try:
            u = lower(spec, ver=ver)
            shas[ver] = DveOpSpec(
                name=_OP_NAME, opcode=row, uops=u, rd1_en=_has_src1(spec)
            ).sha(ver)
        except ValueError:
            pass
    op = dve_ops.DveOp(_OP_NAME, spec, subdim=False, uops_sha=shas)
    dve_ops.OPS.append(op)
    dve_ops._SUB_OPCODE_FOR_NAME[_OP_NAME] = row
    dve_ops.CUSTOM_DVE_SPECS[_OP_NAME] = spec
    return op


def _build_nc():
    import concourse.bass as bass
    import concourse.tile as tile
    from concourse import mybir

    ttop = _register_custom_op()

    f32 = mybir.dt.float32
    bf16 = mybir.dt.bfloat16
    fp16 = mybir.dt.float16

    nc = bass.Bass()
    # cg0 split by contraction half so the two DMA queues race it in first
    # (the first PSUM tile gates the whole DVE-critical pipeline)
    db0kd = [
        nc.dram_tensor(f"db0k{k}", [128, GW], bf16, kind="ExternalInput")
        for k in (0, 1)
    ]
    dbd = [None] + [
        nc.dram_tensor(f"db{c}", [128, 2, GW], bf16, kind="ExternalInput")
        for c in range(1, NCG)
    ]
    nbcd = [
        nc.dram_tensor(f"nbc{c}", [128, GW], fp16, kind="ExternalInput")
        for c in range(NCG)
    ]
    diagwd = nc.dram_tensor("diagw", [128, RT], f32, kind="ExternalInput")
    o = nc.dram_tensor("o", [128, RT, NCG], f32, kind="ExternalOutput")

    with tile.TileContext(nc) as tc:
        with (
            tc.tile_pool(name="sb", bufs=1) as sb,
            tc.tile_pool(name="pp", bufs=1) as pp,
            tc.tile_pool(name="ps", bufs=2, space="PSUM") as ps,
        ):
            # DMA plan: small/critical first; db/nbc chunks land in
            # consumption order, spread over four DGE queues.
            tdw = sb.tile([128, RT], f32, tag="dw")
            nc.gpsimd.dma_start(out=tdw, in_=diagwd[:])
            tnbc = [
                sb.tile([128, GW], fp16, name=f"nbc{c}", tag=f"nbc{c}")
                for c in range(NCG)
            ]
            nc.gpsimd.dma_start(out=tnbc[0], in_=nbcd[0][:])
            qs = [nc.sync, nc.scalar]
            tdb0k = [
                sb.tile([128, GW], bf16, name=f"db0k{k}", tag=f"db0k{k}")
                for k in (0, 1)
            ]
            nc.sync.dma_start(out=tdb0k[0], in_=db0kd[0][:])
            nc.scalar.dma_start(out=tdb0k[1], in_=db0kd[1][:])
            tdb = [None] + [
                sb.tile([128, 2, GW], bf16, name=f"db{c}", tag=f"db{c}")
                for c in range(1, NCG)
            ]
            for c in range(1, NCG):
                qs[c % 2].dma_start(out=tdb[c], in_=dbd[c][:])
            for c in range(1, NCG):
                nc.gpsimd.dma_start(out=tnbc[c], in_=nbcd[c][:])

            # wait absorbers: DVE observes the startup-critical DMAs once
            dum = sb.tile([128, 2], f32, tag="dum")
            nc.vector.tensor_copy(out=dum[:, 0:1], in_=tdw[:, 0:1])
            nc.vector.tensor_copy(out=dum[:, 1:2], in_=tnbc[0][:, 0:1])
            dumo = sb.tile([128, 1], f32, tag="dumo")

            parts = [
                pp.tile([128, NCG], f32, name=f"part{m}", tag=f"part{m}")
                for m in range(RT)
            ]
            for cg in range(NCG):
                for m in range(RT):
                    lhs = [tdb0k[k][:, m * 128 : (m + 1) * 128] for k in (0, 1)]
                    pst = ps.tile([128, GW], f32, tag="pst")
                    for k in (0, 1):
                        for i in (0, 1, 2, 3):
                            rhs = (
                                tdb0k[k][:, i * 512 : (i + 1) * 512]
                                if cg == 0
                                else tdb[cg][:, k, i * 512 : (i + 1) * 512]
                            )
                            nc.tensor.matmul(
                                out=pst[:, i * 512 : (i + 1) * 512],
                                lhsT=lhs[k],
                                rhs=rhs,
                                start=(k == 0),
                                stop=(k == 1),
                            )
                    nc.vector._custom_dve(
                        ttop,
                        out=dumo.broadcast_to(pst.shape),
                        in0=pst,
                        in1=tnbc[cg][:, :],
                        s0=tdw[:, m : m + 1] if cg == 0 else -1.0,
                        s1=0.0,
                        imm2=0.0,
                        accum_out=parts[m][:, cg : cg + 1],
                    )
                    if cg == NCG - 1:
                        # parts[m] is complete — ship it immediately
                        nc.sync.dma_start(out=o[:, m, :], in_=parts[m])

    from concourse.bass import _bass_rust
    from concourse.library_overlay import lower_extended_insts

    lower_extended_insts(nc)  # populate .instr for InstISA subclasses
    _bass_rust.move_matmul_waits_to_ldweights(nc.m)
    _bass_rust.generate_event_semaphores(nc)
    return nc


def _prep_core(s1, s2T_bf, c2_half, c):
    import ml_dtypes

    bf = ml_dtypes.bfloat16
    s1p = np.roll(s1, -c * QPC, axis=0)
    s1pT_bf = np.ascontiguousarray(s1p.T).astype(bf)
    dbx = np.empty((128, 2, 2 * N1), dtype=bf)
    for k in (0, 1):
        dbx[:, k, 0:N1] = s1pT_bf[k * 128 : (k + 1) * 128]
        dbx[:, k, N1 : 2 * N1] = s2T_bf[k * 128 : (k + 1) * 128]
    dbd = {
        f"db{c_}": np.ascontiguousarray(dbx[:, :, c_ * GW : (c_ + 1) * GW])
        for c_ in range(1, NCG)
    }
    dbd["db0k0"] = np.ascontiguousarray(dbx[:, 0, 0:GW])
    dbd["db0k1"] = np.ascontiguousarray(dbx[:, 1, 0:GW])
    # -0.5||y||^2 broadcast rows: rolled s1 side then s2 side, fp16
    c1_64 = -0.5 * np.square(s1p.astype(np.float64)).sum(1)
    crow_half = np.concatenate([c1_64.astype(np.float16), c2_half])
    nbc = np.ascontiguousarray(np.broadcast_to(crow_half[None, :], (128, 2 * N1)))
    for c_ in range(NCG):
        dbd[f"nbc{c_}"] = np.ascontiguousarray(nbc[:, c_ * GW : (c_ + 1) * GW])
    return dbd


def kernel(s1, s2, k):
    assert int(k) == 1
    from concourse.bass_utils import run_bass_kernel_spmd
    import ml_dtypes

    s1 = np.asarray(s1, dtype=np.float32)
    s2 = np.asarray(s2, dtype=np.float32)
    s2T_bf = np.ascontiguousarray(s2.T).astype(ml_dtypes.bfloat16)
    c2_half = (-0.5 * np.square(s2.astype(np.float64)).sum(1)).astype(np.float16)
    diagw = (
        np.arange(RT, dtype=np.float32)[None, :] * 128
        + np.arange(128, dtype=np.float32)[:, None]
    )
    diagw = np.ascontiguousarray(diagw)

    nc = _build_nc()
    in_maps = []
    for c in range(NCORES):
        dbd = _prep_core(s1, s2T_bf, c2_half, c)
        in_maps.append({**dbd, "diagw": diagw})
    import os

    res = run_bass_kernel_spmd(
        nc,
        in_maps,
        core_ids=list(range(NCORES)),
        trace=os.environ.get("KBENCH_TRACE") == "1",
    )
    kernel.last_results = res

    # host epilogue (float64): rho/nu from per-group maxes, then the estimator
    sq1 = np.square(s1.astype(np.float64)).sum(1)
    total = 0.0
    for c in range(NCORES):
        part = res.results[c]["o"].astype(np.float64)  # [128, RT, 8]
        maxA = part[:, :, 0:4].max(axis=2)  # [128, RT]
        maxB = part[:, :, 4:8].max(axis=2)
        idx = np.arange(RT)[None, :] * 128 + np.arange(128)[:, None]
        orig = (c * QPC + idx) % N1
        sqx = sq1[orig]
        rho_sq = sqx - 2.0 * maxA
        nu_sq = sqx - 2.0 * maxB
        rho_sq = np.maximum(rho_sq, 1e-20)
        nu_sq = np.maximum(nu_sq, 1e-20)
        total += 0.5 * (np.log(nu_sq) - np.log(rho_sq)).sum()
    base = np.log(N2 / (N1 - 1))
    return np.float32(base + (D / N1) * total)
